# revision 30
# baseline (speedup 1.0000x reference)
"""GQA forward (B=2,T=2048,D=1024,H=16,KV=4,HD=64) on 8 TRN2 NeuronCores.

Sharding: core c -> (batch b=c//4, kv-group g=c%4). Each core computes the
4 query heads of its kv group against its batch, plus the partial output
projection for its 256 columns of the concat-head activation; the host sums
the 4 per-group partials of each batch (row-parallel out_proj unshard).

v3 pipeline: attention runs in 8 units of (head-pair hp, t-quarter tq=512).
Per s-chunk the unit's scores live in a (128, 1024) PSUM tile =
[headA 512 | headB 512], double-buffered ("sc" tag) so QK of chunk s+1
overlaps the exp of chunk s. PV matmuls are emitted 2 chunks behind QK so a
pending PV never head-blocks the in-order PE queue; the pexp SBUF ring
(bufs=8) provides the slack. All projections accumulate in (128,512) "op"
psum tiles so they never contend with the attention score ring. outproj(tq)
is emitted one unit AFTER its inputs complete so its matmuls wait out the
normalization chain inside another unit's span instead of stalling the PE
(which previously re-throttled HAM to 1.2 GHz every unit). The PV copy-out
runs on ScalarE (between exps) to release the single PV accumulator fast;
normalization muls run on GpSimd, reciprocal on DVE in (8,128) shape via a
reshaping DMA hop.
PSUM budget: sc 2x2 banks + pv 2 + op 2x1 = 8.
"""

import os
import sys

for _p in ("/opt/trn_rl_repo",):
    if _p not in sys.path:
        sys.path.insert(0, _p)

import numpy as np

B, T, D = 2, 2048, 1024
H, KV, HD = 16, 4, 64
REP = H // KV          # 4 query heads per core
GH = REP * HD          # 256 q columns per core
P = 128
SC = T // P            # s-chunks (contraction tiles over sequence)
KC = D // P            # k-chunks over the model dim
NTQ = 4                # t-quarters of 512

SWAP_MASK = [i ^ 1 for i in range(32)]  # pair-swap within each 32-partition quadrant

_MODULE_CACHE = {}
LAST_RESULT = None  # test.py reads exec_time_ns / trace path from here

# squaring-based exp: scores arrive pre-scaled (log2e/256 folded into q_w) so
# e^(score/8) = (2^y)^32 with y tiny; pass 1 is a cubic 2^y (constant term via
# the free One input leaf), pass 2 five squarings. ScalarE consumes the same
# scores with scale=32*ln2. DVE shifts/divide are unimplemented on TRN2 —
# this construction needs only MUL/ADD.
ZSCALE = float(np.log2(np.e) / 256.0)
EXPC1, EXPC2, EXPC3 = 0.6931701704564661, 0.2410405980486783, 0.055298672748014394
DVE_EXP_CHUNKS = ()           # DVE exp disabled: ACT-only won on HW


def _register_dve_exp():
    from concourse import dve_ops
    from concourse.dve_spec import Spec, Src0, One, lower, sq, _has_src1
    from concourse.dve_spec import C0 as sC0, C1 as sC1, C2 as sC2
    from concourse.dve_uop import DveOpSpec

    existing = {op.name: op for op in dve_ops.OPS}
    made = {}

    def add(name, spec):
        if name in existing:
            made[name] = existing[name]
            return
        opcode = dve_ops._CUSTOM_DVE_ROW_BASE + len(dve_ops.OPS)
        uops = lower(spec, ver="v3")
        sha = DveOpSpec(name=name, opcode=opcode, uops=uops,
                        rd1_en=_has_src1(spec)).sha("v3")
        op = dve_ops.DveOp(name, spec, subdim=False, uops_sha={"v3": sha})
        dve_ops.OPS.append(op)
        dve_ops._SUB_OPCODE_FOR_NAME[name] = opcode
        dve_ops.CUSTOM_DVE_SPECS[name] = spec
        made[name] = op

    z = Src0
    body1 = (((z * sC0) + sC1) * z + sC2) * z + One
    ref1 = lambda in0, in1, c0, c1, c2: np.float32(
        ((in0 * np.float32(c0) + np.float32(c1)) * in0 + np.float32(c2)) * in0
        + np.float32(1.0)
    )
    add("EXP2S_POLY_ANT", Spec(body=body1, reference=ref1))

    body2 = sq(sq(sq(sq(sq(Src0)))))
    def ref2(in0, in1, c0, c1, c2):
        q = in0.astype(np.float32)
        for _ in range(5):
            q = (q * q).astype(np.float32)
        return q
    add("EXP2S_SQ32_ANT", Spec(body=body2, reference=ref2))
    return made["EXP2S_POLY_ANT"], made["EXP2S_SQ32_ANT"]


def _build():
    import concourse.tile as tile
    from concourse import mybir
    from concourse.bacc import Bacc

    bf16 = mybir.dt.bfloat16
    f32 = mybir.dt.float32
    AF = mybir.ActivationFunctionType

    nc = Bacc(trn_type="TRN2")
    # xT/qwT arrive host-pre-permuted to (p, c*free) so their DMAs are fully
    # contiguous per partition (the strided (c p) t pattern cost ~10.6us of
    # descriptor generation on the sync engine before any data moved)
    xT_h = nc.dram_tensor("xT", (P, KC * T), bf16, kind="ExternalInput")
    qwT_h = nc.dram_tensor("qwT", (P, KC * GH), bf16, kind="ExternalInput")
    kwT_h = nc.dram_tensor("kwT", (D, HD), bf16, kind="ExternalInput")
    vwT_h = nc.dram_tensor("vwT", (D, HD), bf16, kind="ExternalInput")
    owT_h = nc.dram_tensor("owT", (GH, D), bf16, kind="ExternalInput")
    cos_h = nc.dram_tensor("cosF", (P, T), bf16, kind="ExternalInput")
    sin_h = nc.dram_tensor("sinF", (P, T), bf16, kind="ExternalInput")
    out_h = nc.dram_tensor("outT", (P, KC * T), bf16, kind="ExternalOutput")

    xTr = xT_h[:, :].rearrange("p (c t) -> p c t", t=T)
    qwTr = qwT_h[:, :].rearrange("p (c m) -> p c m", m=GH)
    kwTr = kwT_h[:, :].rearrange("(c p) m -> p c m", p=P)
    vwTr = vwT_h[:, :].rearrange("(c p) m -> p c m", p=P)
    owTr = owT_h[:, :].rearrange("(c p) n -> p c n", p=P)
    outr = out_h[:, :].rearrange("p (c t) -> p c t", t=T)

    act_scale = float(32.0 * np.log(2.0))   # scores are pre-scaled by ZSCALE
    POLY_OP, SQ32_OP = _register_dve_exp()

    with tile.TileContext(nc) as tc:
        with (
            tc.tile_pool(name="consts", bufs=1) as consts,
            tc.tile_pool(name="rope", bufs=3) as rope,
            tc.tile_pool(name="pexp", bufs=8) as pexp,
            tc.tile_pool(name="norm", bufs=2) as norm,
            tc.tile_pool(name="qtmp", bufs=2) as qtmp,
            tc.tile_pool(name="outs", bufs=4) as outs,
            tc.tile_pool(name="ps", bufs=1, space="PSUM") as ps,
        ):
            # ---- loads, in consumption order, one DMA per tensor ----
            kwT_sb = consts.tile([P, KC, HD], bf16)
            nc.sync.dma_start(out=kwT_sb, in_=kwTr)
            x_all = consts.tile([P, KC, T], bf16)
            nc.sync.dma_start(out=x_all, in_=xTr)
            x_sb = [x_all[:, c, :] for c in range(KC)]
            cos_sb = consts.tile([P, T], bf16)
            nc.sync.dma_start(out=cos_sb, in_=cos_h[:, :])
            sin_sb = consts.tile([P, T], bf16)
            nc.sync.dma_start(out=sin_sb, in_=sin_h[:, :])
            qw_all = consts.tile([P, KC, GH], bf16)
            nc.sync.dma_start(out=qw_all, in_=qwTr)
            qw_sb = [qw_all[:, c, :] for c in range(KC)]
            vwT_sb = consts.tile([P, KC, HD], bf16)
            nc.sync.dma_start(out=vwT_sb, in_=vwTr)
            owT_sb = consts.tile([P, 2, D], bf16)
            nc.sync.dma_start(out=owT_sb, in_=owTr)

            warm_sb = consts.tile([P, 512], bf16)
            nc.vector.memset(warm_sb, 0.0)
            wps = ps.tile([P, 512], f32, tag="op", name="wps", bufs=2)
            for _ in range(45):
                nc.tensor.matmul(wps, lhsT=warm_sb[:, 0:P], rhs=warm_sb,
                                 start=True, stop=True)

            qro_sb = consts.tile([P, 2, T], bf16)
            kdup_sb = consts.tile([P, T], bf16)
            v_sb = consts.tile([P, SC, HD + 1], bf16)
            nc.vector.memset(v_sb[:, :, HD : HD + 1], 1.0)
            ot_sb = [
                consts.tile([P, 2, 512], bf16, tag=f"ot{i}", name=f"ot{i}")
                for i in range(NTQ)
            ]

            def rope_tile(ps_ap, out_ap, tsl):
                p_sz = ps_ap.shape[0]
                sw = rope.tile([P, 512], f32, tag="sw")
                nc.vector.stream_shuffle(sw[:p_sz], ps_ap, SWAP_MASK)
                t1 = rope.tile([P, 512], f32, tag="t1")
                nc.vector.tensor_mul(t1[:p_sz], ps_ap, cos_sb[:p_sz, tsl])
                nc.vector.tensor_mul(sw[:p_sz], sw[:p_sz], sin_sb[:p_sz, tsl])
                nc.vector.tensor_add(out_ap, t1[:p_sz], sw[:p_sz])

            # ---- k projection: 4 t-tiles of (64, 512) in the "op" psum tag;
            # the partition-64..127 duplicate is copied per t-tile so the
            # first attention chunks never wait on the full-T copy ----
            def kproj(t):
                tsl = slice(t * 512, (t + 1) * 512)
                kps = ps.tile([HD, 512], f32, tag="op", name="kps", bufs=2)
                for c in range(KC):
                    nc.tensor.matmul(
                        kps,
                        lhsT=kwT_sb[:, c, :],
                        rhs=x_sb[c][:, tsl],
                        start=(c == 0),
                        stop=(c == KC - 1),
                    )
                rope_tile(kps, kdup_sb[0:HD, tsl], tsl)
                nc.vector.tensor_copy(kdup_sb[HD:P, tsl], kdup_sb[0:HD, tsl])

            # ---- q projection for head pair m, one (128,512) tile per tq ----
            def qproj(m, tqs):
                for tq in tqs:
                    tsl = slice(tq * 512, (tq + 1) * 512)
                    qps = ps.tile([P, 512], f32, tag="op", name="qps", bufs=2)
                    for c in range(KC):
                        nc.tensor.matmul(
                            qps,
                            lhsT=qw_sb[c][:, m * P : (m + 1) * P],
                            rhs=x_sb[c][:, tsl],
                            start=(c == 0),
                            stop=(c == KC - 1),
                        )
                    rope_tile(qps, qro_sb[:, m, tsl], tsl)

            # ---- v projection: 8 (128, 64) s-groups per (128,512) "op" tile ----
            def vproj(half):
                vps = ps.tile([P, 512], f32, tag="op", name="vps", bufs=2)
                for idx in range(8):
                    s = half * 8 + idx
                    vsl = slice(idx * HD, (idx + 1) * HD)
                    for c in range(KC):
                        nc.tensor.matmul(
                            vps[:, vsl],
                            lhsT=x_sb[c][:, s * P : (s + 1) * P],
                            rhs=vwT_sb[:, c, :],
                            start=(c == 0),
                            stop=(c == KC - 1),
                        )
                # one strided copy moves all 8 s-groups into v_sb
                nc.vector.tensor_copy(
                    v_sb[:, half * 8 : half * 8 + 8, 0:HD],
                    vps[:, :].rearrange("p (s d) -> p s d", d=HD),
                )

            # minimal set for unit(0,0)'s first chunks, then the rest
            kproj(0)
            qproj(0, [0])
            vproj(0)
            kproj(1)
            kproj(2)
            kproj(3)
            vproj(1)
            qproj(1, [0])

            # ---- attention unit (hp, tq): PV trails QK/exp by 2 chunks ----
            def unit(hp, tq, use_dve=True, tail=False):
                tqsl = slice(tq * 512, (tq + 1) * 512)
                pv = ps.tile([HD + 1, 1024], f32, tag="pv", name="pv", bufs=1)
                pes = [None] * SC

                def qk_exp(s):
                    ssl = slice(s * P, (s + 1) * P)
                    sc_t = ps.tile([P, 1024], f32, tag="sc", name="sc", bufs=2)
                    nc.tensor.matmul(
                        sc_t[:, 0:512],
                        lhsT=kdup_sb[0:HD, ssl],
                        rhs=qro_sb[0:HD, hp, tqsl],
                        start=True, stop=True,
                    )
                    nc.tensor.matmul(
                        sc_t[:, 512:1024],
                        lhsT=kdup_sb[HD:P, ssl],
                        rhs=qro_sb[HD:P, hp, tqsl],
                        start=True, stop=True,
                    )
                    pe = pexp.tile([P, 1024], bf16, tag="p", name="pe")
                    if use_dve and s in DVE_EXP_CHUNKS:
                        qt = qtmp.tile([P, 1024], f32, tag="q", name="qt")
                        nc.vector._custom_dve(
                            POLY_OP, out=qt, in0=sc_t,
                            s0=EXPC3, s1=EXPC2, imm2=EXPC1,
                        )
                        nc.vector._custom_dve(SQ32_OP, out=pe, in0=qt)
                    else:
                        nc.scalar.activation(pe, sc_t, AF.Exp, scale=act_scale)
                    pes[s] = pe

                def pv_mm(s):
                    nc.tensor.matmul(
                        pv[:, 0:512],
                        lhsT=v_sb[:, s, :],
                        rhs=pes[s][:, 0:512],
                        start=(s == 0), stop=(s == SC - 1),
                    )
                    nc.tensor.matmul(
                        pv[:, 512:1024],
                        lhsT=v_sb[:, s, :],
                        rhs=pes[s][:, 512:1024],
                        start=(s == 0), stop=(s == SC - 1),
                    )

                LAG = 2   # PV trails QK/exp so a pending PV never blocks PE
                for s in range(SC):
                    qk_exp(s)
                    if s >= LAG:
                        pv_mm(s - LAG)
                for s in range(SC - LAG, SC):
                    pv_mm(s)

                # copy-out on ScalarE to release pv fast; denom -> recip ->
                # broadcast -> fused muls (GpSimd) entirely off the PE path
                of = norm.tile([HD + 1, 1024], f32, tag="of")
                if tail:
                    nc.scalar.copy(of, pv)
                else:
                    nc.vector.tensor_copy(of, pv)
                dn = norm.tile([8, 128], f32, tag="dn")
                nc.sync.dma_start(out=dn, in_=of[HD : HD + 1, :])
                recip8 = norm.tile([8, 128], f32, tag="recip8")
                nc.vector.reciprocal_approx_fast(recip8, dn)
                recip = norm.tile([1, 1024], f32, tag="recip")
                nc.sync.dma_start(out=recip, in_=recip8)
                rb = norm.tile([HD, 1024], f32, tag="rb")
                nc.gpsimd.partition_broadcast(rb, recip)
                nc.vector.tensor_mul(
                    ot_sb[tq][0:HD, hp, :], of[0:HD, 0:512], rb[:, 0:512]
                )
                nc.vector.tensor_mul(
                    ot_sb[tq][HD:P, hp, :], of[0:HD, 512:1024], rb[:, 512:1024]
                )

            # ---- output projection for one t-quarter ----
            def outproj(tq, tail=False):
                tqsl = slice(tq * 512, (tq + 1) * 512)
                for oc in range(KC):
                    osl = slice(oc * P, (oc + 1) * P)
                    op = ps.tile([P, 512], f32, tag="op", name="op", bufs=2)
                    for c in range(2):
                        nc.tensor.matmul(
                            op,
                            lhsT=owT_sb[:, c, osl],
                            rhs=ot_sb[tq][:, c, :],
                            start=(c == 0),
                            stop=(c == 1),
                        )
                    o_sb = outs.tile([P, 512], bf16, tag="o", name="o_sb")
                    if tail and oc % 2 == 0:
                        nc.scalar.copy(o_sb, op)
                    else:
                        nc.vector.tensor_copy(o_sb, op)
                    nc.sync.dma_start(out=outr[:, oc, tqsl], in_=o_sb)

            # outproj(tq) is emitted two units after its inputs complete: its
            # matmuls depend (via ot_sb) on the tq-units' normalization muls,
            # and its DVE casts sit in the DVE FIFO ahead of later units'
            # reciprocals — one unit of slack is not enough to stop that
            # coupled-FIFO convoy from stalling the PE every unit
            unit(0, 0, use_dve=False)
            qproj(0, [1, 2, 3])
            unit(1, 0, use_dve=False)
            qproj(1, [1, 2, 3])
            unit(0, 1)
            unit(1, 1)
            outproj(0)
            unit(0, 2)
            unit(1, 2)
            outproj(1)
            unit(0, 3)
            unit(1, 3, tail=True)
            outproj(2)
            outproj(3, tail=True)

    nc.finalize()
    return nc


def _get_module():
    if "nc" not in _MODULE_CACHE:
        _MODULE_CACHE["nc"] = _build()
    return _MODULE_CACHE["nc"]


def _host_freqs(freqs_cos, freqs_sin):
    cos = np.asarray(freqs_cos, dtype=np.float32)  # (T, 32)
    sin = np.asarray(freqs_sin, dtype=np.float32)
    c64 = np.repeat(cos, 2, axis=1)                # (T, 64): col d -> cos[t, d//2]
    s64 = np.empty((T, HD), dtype=np.float32)
    s64[:, 0::2] = -sin
    s64[:, 1::2] = sin
    cosF = np.ascontiguousarray(np.concatenate([c64, c64], axis=1).T)  # (128, T)
    sinF = np.ascontiguousarray(np.concatenate([s64, s64], axis=1).T)
    return cosF, sinF


def kernel(x, q_w, kv_w, out_w, freqs_cos, freqs_sin):
    global LAST_RESULT
    import ml_dtypes
    from concourse.bass_utils import run_bass_kernel_spmd

    bf = ml_dtypes.bfloat16
    x = np.asarray(x, dtype=np.float32)
    q_w = np.asarray(q_w, dtype=np.float32)
    kv_w = np.asarray(kv_w, dtype=np.float32)
    out_w = np.asarray(out_w, dtype=np.float32)
    cosF, sinF = _host_freqs(freqs_cos, freqs_sin)

    xT = [
        np.ascontiguousarray(
            x[b].T.reshape(KC, P, T).transpose(1, 0, 2).reshape(P, KC * T)
        ).astype(bf)
        for b in range(B)
    ]
    in_maps = []
    for core in range(8):
        b, g = core // KV, core % KV
        in_maps.append(
            dict(
                xT=xT[b],
                qwT=np.ascontiguousarray(
                    (q_w[g * GH : (g + 1) * GH, :].T * ZSCALE)
                    .reshape(KC, P, GH)
                    .transpose(1, 0, 2)
                    .reshape(P, KC * GH)
                ).astype(bf),
                kwT=np.ascontiguousarray(kv_w[g * HD : (g + 1) * HD, :].T).astype(bf),
                vwT=np.ascontiguousarray(
                    kv_w[(KV + g) * HD : (KV + g + 1) * HD, :].T
                ).astype(bf),
                owT=np.ascontiguousarray(out_w[:, g * GH : (g + 1) * GH].T).astype(bf),
                cosF=cosF.astype(bf),
                sinF=sinF.astype(bf),
            )
        )

    nc = _get_module()
    trace = os.environ.get("KERNEL_TRACE", "0") == "1"
    res = run_bass_kernel_spmd(nc, in_maps, core_ids=list(range(8)), trace=trace)
    LAST_RESULT = res

    out = np.zeros((B, T, D), dtype=np.float32)
    for core in range(8):
        b = core // KV
        r = res.results[core]["outT"].astype(np.float32)
        out[b] += r.reshape(P, KC, T).transpose(1, 0, 2).reshape(D, T).T
    return out
